# revision 1
# baseline (speedup 1.0000x reference)
"""AdditiveAttention via separable sin/tanh approximation — 8 TRN2 cores.

scores[q,k] = sum_h wv_h * tanh(qp_h + kp_h) with qp = q@Wq, kp = k@Wk.
tanh(a+b) is replaced by a fitted separable expansion whose factors are
single ACT ops (tanh/sin with scale+bias) plus cheap DVE/Pool products:
    tanh(a+b) ~= sum_r u_r * F_r(a) * G_r(b) + psi(b) + phi(a)
phi(a) is dropped (softmax row-shift invariance). psi(b) and the valid-length
mask enter through the Exp's per-partition bias (zero extra cost).

Sharding: core c <- batch c//2, query rows (c%2)*128..+128. Graph is built
for nk = ceil(max(valid_lens)/128) key tiles and cached per nk.
"""

import math
import sys

sys.path.insert(0, "/opt/trn_rl_repo")

from contextlib import ExitStack

import numpy as np

import concourse.bass as bass
import concourse.mybir as mybir
from concourse import bass_utils, tile

B, LQ, LK, DQ, DK, DV, H = 4, 256, 512, 256, 256, 256, 128
NCORES = 8
F32 = mybir.dt.float32
BF16 = mybir.dt.bfloat16
AF = mybir.ActivationFunctionType

# ---------------------------------------------------------------- fitted model
# Placeholder config (overwritten by gen_cfg output pasted below).
CFG = {
    "q_env": 0.472859,
    "k_env": 0.298637,
    "q_units": {'s0': ('SinT', 1.743912, 1.337396), 's1': ('SinT', -0.106208, 1.059429), 's2': ('SinT', 2.025988, 0.098625), 's3': ('SinT', -2.091873, 0.608901), 's4': ('SinT', 2.559096, 0.41656), 's5': ('SinT', -1.318718, 1.136896)},
    "k_units": {'t0': ('Tanh', 1.046613, -0.175847), 's0': ('SinT', 0.810499, 0.09806), 's1': ('SinT', -1.190885, 0.934057), 's2': ('SinT', 2.432152, 0.1403), 's3': ('SinT', -2.267583, 0.633108)},
    "chunks": [('s0*s1*s5', 's1*s1', -3.571346), ('s0*s2', 'b*t0', -0.241396), ('s1*s2*s2', 's0*s0*s3', -2.500278), ('s1*s1*s3', 'b2', -0.014517), ('s1*s2*s5', 's1*s2*s3', 1.155056), ('s0*s2*s3', 's2*s2*s2', -1.103035), ('s2*s2*s5', 't0*s2', -0.486392), ('s2*s2*s2', 's2*s3*s3', -0.490233), ('s0*s3*s5', 's0*s2*s3', 2.028046), ('s2*s2*s3', 'b*s3', -0.122238), ('s2*s2*s2', 's0*s0*s1', -2.582956), ('a2', 's3*s3', -0.017979), ('s0*s0*s3', 's0*s0*s1', -3.065711), ('a*s1', 'b2', -0.006275), ('a', 's1*s2', -0.054732), ('s4*s4*s4', 's0*s0*s2', -1.189488)],
    "psi": [],
}
# ------------------------------------------------------------------------------


def _q_needs(cfg):
    units, lin, sq = set(), False, False
    for qc, _, _ in cfg["chunks"]:
        for part in qc.split("*"):
            if part == "a":
                lin = True
            elif part == "a2":
                sq = True
            else:
                units.add(part)
    return units, lin, sq


def _k_needs(cfg):
    units, lin, sq = set(), False, False
    cols = [kc for _, kc, _ in cfg["chunks"]] + [kc for kc, _ in cfg["psi"]]
    for kc in cols:
        for part in kc.split("*"):
            if part == "b":
                lin = True
            elif part == "b2":
                sq = True
            else:
                units.add(part)
    return units, lin, sq


def _body(ctx: ExitStack, tc: "tile.TileContext", aps: dict, nk: int):
    cfg = CFG
    nc = tc.nc
    LKe = 128 * nk
    nch = len(cfg["chunks"])
    npsi = len(cfg["psi"])
    pool = ctx.enter_context(tc.tile_pool(name="p", bufs=1))
    ps = ctx.enter_context(tc.tile_pool(name="ps", bufs=1, space="PSUM"))

    # ---- input DMAs (hoisted into the preamble post-build) ----
    # blobA (SP): wk(256) | kin(2*LKe) | wq(256) | qin(256)
    acols = 768 + 2 * LKe
    blobA = pool.tile([128, acols], BF16, tag="blobA", name="blobA")
    nc.sync.dma_start(blobA[:], aps["blobA"][:, :])
    # blobB (ACT): mv tiles (nk*257) | psi rhs cols (npsi)
    bcols = nk * 257 + npsi
    blobB = pool.tile([128, bcols], BF16, tag="blobB", name="blobB")
    nc.scalar.dma_start(blobB[:], aps["blobB"][:, :])
    # blobC (ACT): wr chunk scales (nch) | maskbias (nk) | unit biases  [f32]
    qun_all = sorted(_q_needs(cfg)[0])
    kun_all = sorted(_k_needs(cfg)[0])
    ccols = nch + nk + len(qun_all) + len(kun_all)
    blobC = pool.tile([128, ccols], F32, tag="blobC", name="blobC")
    nc.scalar.dma_start(blobC[:], aps["blobC"][:, :])
    qbias = {un: blobC[:, nch + nk + i : nch + nk + i + 1]
             for i, un in enumerate(qun_all)}
    kbias = {un: blobC[:, nch + nk + len(qun_all) + i : nch + nk + len(qun_all) + i + 1]
             for i, un in enumerate(kun_all)}

    wk = blobA[:, 0:256]
    kin = blobA[:, 256 : 256 + 2 * LKe]
    wq = blobA[:, 256 + 2 * LKe : 512 + 2 * LKe]
    qin = blobA[:, 512 + 2 * LKe : 768 + 2 * LKe]

    # ---- ACT activation-table warm (no data deps; runs during DMA wait) ----
    warm = pool.tile([128, 8], BF16, tag="warm", name="warm")
    nc.vector.memset(warm[:], 0.0)
    nc.scalar.activation(warm[:, 0:1], warm[:, 0:1], AF.Tanh, bias=0.0, scale=1.0)

    # ---- projections ----
    kproj = ps.tile([128, LKe], F32, tag="kproj", name="kproj")
    nc.tensor.matmul(kproj[:], lhsT=wk[:, 0:128], rhs=kin[:, 0:LKe],
                     start=True, stop=False)
    nc.tensor.matmul(kproj[:], lhsT=wk[:, 128:256], rhs=kin[:, LKe : 2 * LKe],
                     start=False, stop=True)
    qproj = ps.tile([128, 128], F32, tag="qproj", name="qproj")
    nc.tensor.matmul(qproj[:], lhsT=wq[:, 0:128], rhs=qin[:, 0:128],
                     start=True, stop=False)
    nc.tensor.matmul(qproj[:], lhsT=wq[:, 128:256], rhs=qin[:, 128:256],
                     start=False, stop=True)

    # ---- k-side tiles ----
    kun, klin, ksq = _k_needs(cfg)
    ktile = {}
    qun_pre = _q_needs(cfg)[0]
    if any(cfg["k_units"][un][0] == "SinT" for un in kun):
        kenv = pool.tile([128, LKe], BF16, tag="kenv", name="kenv")
        nc.scalar.activation(kenv[:], kproj[:], AF.Tanh, bias=0.0,
                             scale=float(cfg["k_env"]))
    qenv = None
    if any(cfg["q_units"][un][0] == "SinT" for un in qun_pre):
        qenv = pool.tile([128, 128], BF16, tag="qenv", name="qenv")
        nc.scalar.activation(qenv[:], qproj[:], AF.Tanh, bias=0.0,
                             scale=float(cfg["q_env"]))
    for un in sorted(kun, key=lambda u: (cfg["k_units"][u][0] == "SinT", u)):
        fn, sc, bi = cfg["k_units"][un]
        t = pool.tile([128, LKe], BF16, tag=f"k_{un}", name=f"k_{un}")
        if fn == "SinT":
            nc.scalar.activation(t[:], kenv[:], AF.Sin, bias=kbias[un],
                                 scale=float(sc))
        else:
            nc.scalar.activation(t[:], kproj[:], getattr(AF, fn), bias=kbias[un],
                                 scale=float(sc))
        ktile[un] = t
    if klin or ksq:
        kb = pool.tile([128, LKe], BF16, tag="kb", name="kb")
        nc.vector.tensor_copy(kb[:], kproj[:])
        ktile["b"] = kb
    if ksq:
        t = pool.tile([128, LKe], BF16, tag="kb2", name="kb2")
        nc.vector.tensor_tensor(out=t[:], in0=ktile["b"][:], in1=ktile["b"][:],
                                op=mybir.AluOpType.mult)
        ktile["b2"] = t

    kcol = {}
    kprod_engine = [nc.vector, nc.gpsimd]
    kprod_i = 0

    def get_kcol(name):
        if name in kcol:
            return kcol[name]
        nonlocal kprod_i
        if "*" not in name:
            kcol[name] = ktile[name]
            return kcol[name]
        parts = sorted(name.split("*"))
        key = "*".join(parts)
        if key in kcol:
            kcol[name] = kcol[key]
            return kcol[name]
        t = pool.tile([128, LKe], BF16, tag=f"kc_{key}",
                      name=f"kc_{''.join(parts)}")
        eng = kprod_engine[kprod_i % 2]
        kprod_i += 1
        if len(parts) == 2:
            eng.tensor_tensor(out=t[:], in0=ktile[parts[0]][:],
                              in1=ktile[parts[1]][:], op=mybir.AluOpType.mult)
        else:
            # choose the pairing whose base pair is already cached
            cands = [(parts[0], parts[1], parts[2]),
                     (parts[0], parts[2], parts[1]),
                     (parts[1], parts[2], parts[0])]
            best = None
            for x, y, z in cands:
                if "*".join(sorted([x, y])) in kcol:
                    best = (x, y, z)
                    break
            if best is None:
                best = cands[0]
            x, y, z = best
            base = get_kcol(f"{x}*{y}")
            eng.tensor_tensor(out=t[:], in0=base[:], in1=ktile[z][:],
                              op=mybir.AluOpType.mult)
        kcol[key] = t
        kcol[name] = t
        return t

    # ---- q-side tiles ----
    qun, qlin, qsq = _q_needs(cfg)
    qtile = {}
    for un in sorted(qun, key=lambda u: (cfg["q_units"][u][0] == "SinT", u)):
        fn, sc, bi = cfg["q_units"][un]
        t = pool.tile([128, 128], BF16, tag=f"q_{un}", name=f"q_{un}")
        if fn == "SinT":
            nc.scalar.activation(t[:], qenv[:], AF.Sin, bias=qbias[un],
                                 scale=float(sc))
        else:
            nc.scalar.activation(t[:], qproj[:], getattr(AF, fn), bias=qbias[un],
                                 scale=float(sc))
        qtile[un] = t
    needs_qb = qlin
    if needs_qb:
        qb = pool.tile([128, 128], BF16, tag="qb", name="qb")
        nc.vector.tensor_copy(qb[:], qproj[:])
        qtile["a"] = qb
    if qsq:
        t = pool.tile([128, 128], BF16, tag="qa2", name="qa2")
        nc.scalar.activation(t[:], qproj[:], AF.Square, bias=0.0, scale=1.0)
        qtile["a2"] = t

    # cached unscaled q columns, then one scale per chunk (DVE/ACT alternating)
    qcol = {}
    qprod_i = [0]

    def get_qcol(name):
        if name in qcol:
            return qcol[name]
        if "*" not in name:
            qcol[name] = qtile[name]
            return qcol[name]
        parts = sorted(name.split("*"))
        key = "*".join(parts)
        if key in qcol:
            qcol[name] = qcol[key]
            return qcol[name]
        t = pool.tile([128, 128], BF16, tag=f"qc_{key}",
                      name=f"qc_{''.join(parts)}")
        if len(parts) == 2:
            nc.vector.tensor_tensor(out=t[:], in0=qtile[parts[0]][:],
                                    in1=qtile[parts[1]][:],
                                    op=mybir.AluOpType.mult)
        else:
            cands = [(parts[0], parts[1], parts[2]),
                     (parts[0], parts[2], parts[1]),
                     (parts[1], parts[2], parts[0])]
            best = None
            for x, y, z in cands:
                if "*".join(sorted([x, y])) in qcol:
                    best = (x, y, z)
                    break
            if best is None:
                best = cands[0]
            x, y, z = best
            base = get_qcol(f"{x}*{y}")
            nc.vector.tensor_tensor(out=t[:], in0=base[:], in1=qtile[z][:],
                                    op=mybir.AluOpType.mult)
        qcol[key] = t
        qcol[name] = t
        return t

    qchunk = []
    for r, (qc, kc, u) in enumerate(cfg["chunks"]):
        wr = blobC[:, r : r + 1]
        out = pool.tile([128, 128], BF16, tag=f"Q{r}", name=f"Q{r}")
        colt = get_qcol(qc)
        if qprod_i[0] % 2 == 0:
            nc.vector.tensor_scalar_mul(out[:], colt[:], wr)
        else:
            nc.scalar.activation(out[:], colt[:], AF.Copy, bias=0.0, scale=wr)
        qprod_i[0] += 1
        qchunk.append(out)

    # ---- scores: nk PSUM tiles [k,q] accumulated over chunks ----
    # order chunks by k-col readiness: direct cols first, products later
    order = sorted(range(nch), key=lambda r: ("*" in cfg["chunks"][r][1],))
    sc_ps = [ps.tile([128, 128], F32, tag=f"sc{t}", name=f"sc{t}") for t in range(nk)]
    for t in range(nk):
        for i, r in enumerate(order):
            qc, kc, u = cfg["chunks"][r]
            g = get_kcol(kc)
            nc.tensor.matmul(sc_ps[t][:], lhsT=g[:, 128 * t : 128 * (t + 1)],
                             rhs=qchunk[r][:], start=(i == 0), stop=(i == nch - 1))

    # ---- psi column + mask bias ----
    if npsi:
        psi_ps = ps.tile([128, max(nk, 2)], F32, tag="psi", name="psi_ps")
        for t in range(nk):
            for i, (kc, w) in enumerate(cfg["psi"]):
                g = get_kcol(kc)
                nc.tensor.matmul(
                    psi_ps[:, t : t + 1], lhsT=g[:, 128 * t : 128 * (t + 1)],
                    rhs=blobB[:, nk * 257 + i : nk * 257 + i + 1],
                    start=(i == 0), stop=(i == npsi - 1))
        psi_sb = pool.tile([128, max(nk, 2)], F32, tag="psi_sb", name="psi_sb")
        nc.vector.tensor_tensor(out=psi_sb[:, 0:nk], in0=psi_ps[:, 0:nk],
                                in1=blobC[:, nch : nch + nk],
                                op=mybir.AluOpType.add)
    else:
        psi_sb = blobC[:, nch : nch + nk]

    # exp-table warm: no deps on scores; issues right after unit ACT ops
    nc.scalar.activation(warm[:, 1:2], warm[:, 1:2], AF.Exp, bias=0.0, scale=1.0)

    # ---- softmax numerator/denominator ----
    pT = []
    for t in range(nk):
        x = pool.tile([128, 128], BF16, tag=f"pT{t}", name=f"pT{t}")
        nc.scalar.activation(x[:], sc_ps[t][:], AF.Exp,
                             bias=psi_sb[:, t : t + 1], scale=1.0)
        pT.append(x)

    out_ps = ps.tile([128, 257], F32, tag="qproj", name="out_ps")  # reuse bank
    for t in range(nk):
        nc.tensor.matmul(out_ps[:], lhsT=pT[t][:],
                         rhs=blobB[:, 257 * t : 257 * (t + 1)],
                         start=(t == 0), stop=(t == nk - 1))

    out_sb = pool.tile([128, 257], F32, tag="out_sb", name="out_sb")
    nc.vector.tensor_copy(out_sb[:], out_ps[:])
    nc.sync.dma_start(aps["out"][:, :], out_sb[:])


def build_graph(nk: int) -> bass.Bass:
    nc = bass.Bass("TRN2", target_bir_lowering=False, debug=False)
    LKe = 128 * nk
    nch = len(CFG["chunks"])
    npsi = len(CFG["psi"])
    aps = {
        "blobA": nc.dram_tensor("blobA", [128, 768 + 2 * LKe], BF16,
                                kind="ExternalInput").ap(),
        "blobB": nc.dram_tensor("blobB", [128, nk * 257 + npsi], BF16,
                                kind="ExternalInput").ap(),
        "blobC": nc.dram_tensor(
            "blobC",
            [128, nch + nk + len(sorted(_q_needs(CFG)[0])) + len(sorted(_k_needs(CFG)[0]))],
            F32, kind="ExternalInput").ap(),
        "out": nc.dram_tensor("out", [128, 257], F32, kind="ExternalOutput").ap(),
    }
    with tile.TileContext(nc) as tc:
        with ExitStack() as ctx:
            _body(ctx, tc, aps, nk)
    _split_multi_waits(nc)
    _hoist_input_dmas(nc)
    return nc


def _split_multi_waits(nc):
    """Walrus accepts only ONE sync-wait per instruction; hoist extras onto
    same-engine NOPs placed immediately before (identical semantics)."""
    n = 0
    for bb in nc.m.functions[0].blocks:
        out = []
        for inst in bb.instructions:
            si = inst.sync_info
            if si is not None and si.on_wait and len(si.on_wait) > 1:
                waits = list(si.on_wait)
                for w in waits[:-1]:
                    nop = mybir.InstNoOp(
                        name=f"{inst.name}-wsplit{n}", text_hint="waitsplit",
                        bass_nofuse=True, engine=inst.engine,
                        sync_info=mybir.SyncInfo(on_wait=[w], on_update=[]))
                    nc.register_instruction(nop)
                    out.append(nop)
                    n += 1
                inst.sync_info = mybir.SyncInfo(on_wait=[waits[-1]],
                                                on_update=si.on_update)
            out.append(inst)
        if n:
            bb.instructions = out


def _hoist_input_dmas(nc):
    """Move waitless input DMACopies into block 0 (after the engine register
    preamble, before the startup barrier) so transfers overlap the barrier."""
    blocks = nc.m.functions[0].blocks
    b0 = blocks[0]
    moved = []
    for bb in blocks[1:]:
        keep = []
        for inst in bb.instructions:
            si = inst.sync_info
            if (type(inst).__name__ == "InstDMACopy"
                    and (si is None or not si.on_wait)):
                moved.append(inst)
            else:
                keep.append(inst)
        bb.instructions = keep
        break
    if not moved:
        return
    insts = list(b0.instructions)
    out, inserted = [], False
    for i, inst in enumerate(insts):
        out.append(inst)
        if not inserted:
            nxt = insts[i + 1] if i + 1 < len(insts) else None
            if (type(inst).__name__ == "InstRegisterMove"
                    and (nxt is None or type(nxt).__name__ != "InstRegisterMove")):
                out.extend(moved)
                inserted = True
    if not inserted:
        out = moved + out
    b0.instructions = out


def make_in_maps(queries, keys, values, Wq, Wk, wv, valid_lens, nk):
    import ml_dtypes

    bf = ml_dtypes.bfloat16
    f = np.float32
    LKe = 128 * nk
    nch = len(CFG["chunks"])
    npsi = len(CFG["psi"])
    queries = np.asarray(queries, f)
    keys = np.asarray(keys, f)
    values = np.asarray(values, f)
    Wqf = np.asarray(Wq, f)
    Wkf = np.asarray(Wk, f)
    wvf = np.asarray(wv, f).reshape(H)

    wk_blob = np.concatenate([Wkf[0:128], Wkf[128:256]], axis=1)
    wq_blob = np.concatenate([Wqf[0:128], Wqf[128:256]], axis=1)

    in_maps = []
    for c in range(NCORES):
        b, half = c // 2, c % 2
        kT = keys[b, 0:LKe].T
        A = np.empty((128, 768 + 2 * LKe), f)
        A[:, 0:256] = wk_blob
        A[:, 256 : 256 + LKe] = kT[0:128]
        A[:, 256 + LKe : 256 + 2 * LKe] = kT[128:256]
        A[:, 256 + 2 * LKe : 512 + 2 * LKe] = wq_blob
        qT = queries[b, 128 * half : 128 * (half + 1), :].T
        A[:, 512 + 2 * LKe : 640 + 2 * LKe] = qT[0:128]
        A[:, 640 + 2 * LKe : 768 + 2 * LKe] = qT[128:256]

        Bb = np.empty((128, nk * 257 + npsi), f)
        for t in range(nk):
            sl = slice(128 * t, 128 * (t + 1))
            Bb[:, 257 * t : 257 * t + 256] = values[b, sl, :]
            Bb[:, 257 * t + 256] = 1.0
        for i, (kc, w) in enumerate(CFG["psi"]):
            Bb[:, nk * 257 + i] = wvf * w

        qun_all = sorted(_q_needs(CFG)[0])
        kun_all = sorted(_k_needs(CFG)[0])
        Cc = np.zeros((128, nch + nk + len(qun_all) + len(kun_all)), f)
        for r, (qc, kc, u) in enumerate(CFG["chunks"]):
            Cc[:, r] = wvf * u
        m = np.arange(LKe) < int(valid_lens[b])
        mb = np.where(m, 0.0, -1e6).astype(f)
        for t in range(nk):
            Cc[:, nch + t] = mb[128 * t : 128 * (t + 1)]
        for i, un in enumerate(qun_all):
            Cc[:, nch + nk + i] = CFG["q_units"][un][2]
        for i, un in enumerate(kun_all):
            Cc[:, nch + nk + len(qun_all) + i] = CFG["k_units"][un][2]

        in_maps.append({"blobA": A.astype(bf), "blobB": Bb.astype(bf), "blobC": Cc})
    return in_maps


_CACHE: dict = {}


def kernel(queries, keys, values, Wq, Wk, wv, valid_lens, _trace=False,
           _trace_kwargs=None):
    nk = min(4, max(1, math.ceil(int(np.max(np.asarray(valid_lens))) / 128)))
    if nk not in _CACHE:
        _CACHE[nk] = build_graph(nk)
    nc = _CACHE[nk]
    in_maps = make_in_maps(queries, keys, values, Wq, Wk, wv, valid_lens, nk)
    res = bass_utils.run_bass_kernel_spmd(
        nc, in_maps, core_ids=list(range(NCORES)), trace=_trace,
        **(_trace_kwargs or {}))
    out = np.empty((B, LQ, DV), dtype=np.float32)
    for c in range(NCORES):
        b, half = c // 2, c % 2
        o = res.results[c]["out"]
        out[b, 128 * half : 128 * (half + 1), :] = o[:, 0:256] / o[:, 256:257]
    if _trace:
        return out, res
    return out

